# revision 39
# baseline (speedup 1.0000x reference)
"""Distributed Trainium2 (Bass/Tile) kernel for masked GAT-style attention.

Reference computation (H=4 heads, N=4096 nodes, D=128):
    scores = leaky_relu(x @ W^T + b, 0.2)            # [H, N, N]
    att    = where(mask, softmax(where(mask, scores, -inf)), 0)
    out    = att @ x                                  # [H, N, D]

Sharding: 8 cores = 4 heads x 2 row-blocks of 2048 nodes. Each core
computes out[h, r0:r0+2048] independently (no collectives).

Per-core layout ("transposed scores"): scores^T tiles [m=128 part, n free]
so the PV matmul uses the attention tile directly as the stationary
operand and the softmax row-sum comes for free from an appended
ones-column on x.

The per-element work is split across engines by m-tile:
- ACT path (m % 3 == 0): leaky_relu via the Prelu LUT, exp, then a
  bf16 mask multiply on VectorE (2x mode).
- DVE path (otherwise): one scalar_tensor_tensor computes
  t = 0.2*s + (-30)*(1-mask) (the additive term folds the mask in:
  masked-out entries end up as exp(0.2*s - 30) ~ 1e-13), a second STT
  computes l = max(5t, t) = leaky(s) - 150*(1-mask), then exp on ACT.
  No post-exp mask multiply needed.
The mask is shipped once as uint16: bf16 1.0/0.0 rows for ACT-path
tiles, fp16 -30*(1-mask) rows for DVE-path tiles, bitcast on chip.
"""

import sys

if "/opt/trn_rl_repo" not in sys.path:
    sys.path.insert(0, "/opt/trn_rl_repo")

import numpy as np
import ml_dtypes

import concourse.bass as bass
import concourse.tile as tile
from concourse import bacc, mybir
from concourse.bass_utils import run_bass_kernel_spmd

BF = mybir.dt.bfloat16
F16 = mybir.dt.float16
F32 = mybir.dt.float32
U16 = mybir.dt.uint16
BF_NP = ml_dtypes.bfloat16

H, N, D = 4, 4096, 128
N_CORES = 8
ROWS = N * H // N_CORES          # 2048 rows (n) per core
CHUNK = 1024                     # n columns processed per outer chunk
CHUNKS = ROWS // CHUNK           # 2
M_TILES = N // 128               # 32 tiles along the softmax (m) axis
SUBS = CHUNK // 128              # 8 PV subtiles per chunk
M_GROUP = 8                      # m-tiles per split constant tile
MB = 4                           # m-tiles per batched mask DMA

# Module-level knobs used by test.py; harmless defaults for grading.
TRACE = False
LAST_EXEC_NS = None

_CACHED_NC = None
_CACHED_BIAS = None


def _on_act(m, has_bias):
    return has_bias or m % 2 == 0


def _build_nc(has_bias=False):
    nc = bacc.Bacc("TRN2", target_bir_lowering=False, debug=False,
                   num_devices=N_CORES)
    xt_d = nc.dram_tensor("xt", [128, ROWS], BF, kind="ExternalInput").ap()
    wt_d = nc.dram_tensor("wt", [128, N], BF, kind="ExternalInput").ap()
    xa_d = nc.dram_tensor("xa", [N, D + 1], BF, kind="ExternalInput").ap()
    mk_d = nc.dram_tensor("mk", [N, ROWS], U16, kind="ExternalInput").ap()
    bc_d = nc.dram_tensor("bc", [128, M_TILES], F32, kind="ExternalInput").ap()
    out_d = nc.dram_tensor("out", [ROWS, D], F32, kind="ExternalOutput").ap()

    PRELU = mybir.ActivationFunctionType.Prelu
    EXP = mybir.ActivationFunctionType.Exp
    COPY = mybir.ActivationFunctionType.Copy
    MUL = mybir.AluOpType.mult
    ADD = mybir.AluOpType.add
    MAX = mybir.AluOpType.max

    n_wt = N // (M_GROUP * 128)      # 4 stationary groups
    n_xa = M_TILES // M_GROUP        # 4 PV-moving groups

    with tile.TileContext(nc) as tc:
        with (
            tc.tile_pool(name="const", bufs=1) as cpool,
            tc.tile_pool(name="mask", bufs=3) as mpool,
            tc.tile_pool(name="work", bufs=6) as wpool,
            tc.tile_pool(name="outp", bufs=3) as opool,
            tc.tile_pool(name="spsum", bufs=2, space="PSUM") as spool,
            tc.tile_pool(name="opsum", bufs=1, space="PSUM") as oppool,
        ):
            # Constants, split into group tiles so the first QK/PV tiles
            # only depend on the first pieces.
            GW = M_GROUP * 128
            wt_sb = [cpool.tile([128, GW], BF, name=f"wt{i}")
                     for i in range(n_wt)]
            xt_sb = [cpool.tile([128, CHUNK], BF, name=f"xt{i}")
                     for i in range(CHUNKS)]
            bc_sb = cpool.tile([128, M_TILES], F32)
            xa_sb = [cpool.tile([128, M_GROUP, D + 1], BF, name=f"xa{i}")
                     for i in range(n_xa)]

            # Warm the ACT LUT set while input DMAs are in flight: a tiny
            # dependency-free activation forces the table load up front.
            warm_sb = cpool.tile([1, 1], F32)
            nc.scalar.activation(warm_sb[:], warm_sb[:], EXP)

            nc.sync.dma_start(out=wt_sb[0][:], in_=wt_d[:, 0:GW])
            nc.sync.dma_start(out=xt_sb[0][:], in_=xt_d[:, 0:CHUNK])
            nc.sync.dma_start(out=bc_sb[:], in_=bc_d[:, :])
            # xa: one batched DMA per group, issued from the idle GpSimd
            # sequencer so they don't delay mask issues on Sync.
            xa_v = xa_d.rearrange("(g j p) d -> g p j d", p=128, j=M_GROUP)
            for i in range(1, n_wt):
                nc.sync.dma_start(out=wt_sb[i][:], in_=wt_d[:, i * GW:(i + 1) * GW])
            for i in range(1, CHUNKS):
                nc.sync.dma_start(out=xt_sb[i][:],
                                  in_=xt_d[:, i * CHUNK:(i + 1) * CHUNK])

            for c in range(CHUNKS):
                # 8 accumulator subtiles of [128, 129] packed 3-per-bank.
                o_ps = [
                    oppool.tile([128, 512], F32, tag=f"oacc{b}",
                                name=f"oacc{b}_c{c}")
                    for b in range((SUBS + 2) // 3)
                ]

                def o_ap(s):
                    return o_ps[s // 3][:, (s % 3) * 129:(s % 3) * 129 + 129]

                mkb_sb = None
                for quad in range(M_TILES // 4):
                    # l/e quad tiles: one 4096-wide exp per 4 m-tiles; the
                    # two ACT halves (j=0,2) share one strided mask multiply.
                    lq_sb = wpool.tile([128, 4, CHUNK], F16, tag="lq",
                                       name=f"lq_c{c}_q{quad}")
                    for j in range(4):
                        m = quad * 4 + j
                        # Mask tiles: one batched DMA per MB m-tiles, issued
                        # from the GpSimd sequencer.
                        if m % MB == 0:
                            mkb_sb = mpool.tile([128, MB, CHUNK], U16,
                                                tag="mkb",
                                                name=f"mkb_c{c}_m{m}")
                            mk_v = mk_d[m * 128:(m + MB) * 128,
                                        c * CHUNK:(c + 1) * CHUNK].rearrange(
                                            "(t p) n -> p t n", p=128)
                            nc.gpsimd.dma_start(out=mkb_sb[:], in_=mk_v)
                            if c == 0 and m == 0:
                                for g in range(n_xa):
                                    nc.gpsimd.dma_start(out=xa_sb[g][:],
                                                        in_=xa_v[g])
                        mk_sb = mkb_sb[:, m % MB]

                        # scores^T tile: [m=128, n=CHUNK] (two 512-col
                        # matmuls, one per PSUM bank).
                        s_ps = spool.tile([128, CHUNK], F32, tag="s",
                                          name=f"s_c{c}_m{m}")
                        wt_g = wt_sb[m // M_GROUP]
                        wcol = (m % M_GROUP) * 128
                        for half in range(CHUNK // 512):
                            nc.tensor.matmul(
                                s_ps[:, half * 512:(half + 1) * 512],
                                lhsT=wt_g[:, wcol:wcol + 128],
                                rhs=xt_sb[c][:, half * 512:(half + 1) * 512],
                                start=True, stop=True,
                            )

                        l_half = lq_sb[:, j]
                        if _on_act(m, has_bias):
                            # ACT path: Prelu LUT (+ per-partition bias).
                            nc.scalar.activation(l_half, s_ps[:], PRELU,
                                                 bias=bc_sb[:, m:m + 1],
                                                 scale=1.0, alpha=0.2)
                        else:
                            # DVE path: t = 0.2*s + (-30)*(1-mask)  [one
                            # STT, single PSUM read], l = max(5t, t).
                            # Mask already folded in.
                            t_sb = wpool.tile([128, CHUNK], F16, tag="lv_t",
                                              name=f"t_c{c}_m{m}")
                            nc.vector.scalar_tensor_tensor(
                                t_sb[:], s_ps[:], 0.2, mk_sb.bitcast(F16),
                                op0=MUL, op1=ADD)
                            nc.vector.scalar_tensor_tensor(
                                l_half, t_sb[:], 5.0, t_sb[:],
                                op0=MUL, op1=MAX)

                    # One exp covers the whole quad.
                    eq_sb = wpool.tile([128, 4, CHUNK], BF, tag="eq",
                                       name=f"eq_c{c}_q{quad}")
                    nc.scalar.activation(eq_sb[:], lq_sb[:], EXP)

                    if not has_bias:
                        # One strided multiply masks both ACT halves
                        # (j = 0 and 2); mask batch MB=4 is quad-aligned.
                        aq_sb = wpool.tile([128, 2, CHUNK], BF, tag="aq",
                                           name=f"aq_c{c}_q{quad}")
                        mkq = mkb_sb[:, 0:MB:2]
                        nc.vector.tensor_mul(aq_sb[:], eq_sb[:, 0:4:2],
                                             mkq.bitcast(BF))

                    for j in range(4):
                        m = quad * 4 + j
                        if _on_act(m, has_bias):
                            if has_bias:
                                a_sb = wpool.tile([128, CHUNK], BF, tag="a",
                                                  name=f"a_c{c}_m{m}")
                                nc.vector.tensor_mul(
                                    a_sb[:], eq_sb[:, j],
                                    mkb_sb[:, m % MB].bitcast(BF))
                                lhs_t = a_sb[:]
                            else:
                                lhs_t = aq_sb[:, j // 2]
                        else:
                            lhs_t = eq_sb[:, j]

                        # PV: out[n, 0:128] += att^T.T @ x ; col 128 =
                        # row-sum. start=True clears has_written for the
                        # WHOLE bank, so only the first sub-chain of each
                        # bank may issue it.
                        for s in range(SUBS):
                            nc.tensor.matmul(
                                o_ap(s),
                                lhsT=lhs_t[:, s * 128:(s + 1) * 128],
                                rhs=xa_sb[m // M_GROUP][:, m % M_GROUP],
                                start=(m == 0 and s % 3 == 0),
                                stop=(m == M_TILES - 1),
                                skip_group_check=True,
                            )

                # Division tail: reciprocal on DVE (tiny), the scale-copy
                # split across ACT and DVE, all results gathered into one
                # tile so the chunk needs a single output DMA.
                of_big = opool.tile([128, SUBS, D], F32, tag="ofbig",
                                    name=f"ofbig_c{c}")
                for s in range(SUBS):
                    ob = o_ap(s)
                    r_sb = opool.tile([128, 1], F32, tag="recip",
                                      name=f"recip_c{c}_s{s}")
                    nc.vector.reciprocal(r_sb[:], ob[:, 128:129])
                    if s % 2 == 0:
                        nc.scalar.activation(of_big[:, s], ob[:, 0:D], COPY,
                                             bias=0.0, scale=r_sb[:])
                    else:
                        nc.vector.tensor_scalar_mul(of_big[:, s], ob[:, 0:D],
                                                    r_sb[:])
                out_v = out_d[c * CHUNK:(c + 1) * CHUNK, :].rearrange(
                    "(s p) d -> p s d", p=128)
                nc.sync.dma_start(out=out_v, in_=of_big[:])

    nc.compile()
    return nc


def _pack_mask(mask_t_u8, has_bias):
    """mask_t_u8: [N, ROWS] 0/1. Returns uint16-packed per-m-tile rows."""
    out = np.empty(mask_t_u8.shape, np.uint16)
    for m in range(M_TILES):
        rows = slice(m * 128, (m + 1) * 128)
        blk = mask_t_u8[rows]
        if _on_act(m, has_bias):
            out[rows] = blk.astype(BF_NP).view(np.uint16)
        else:
            out[rows] = (30.0 * (blk.astype(np.float32) - 1.0)).astype(
                np.float16).view(np.uint16)
    return out


def kernel(x, W, b, neighbor_mask):
    global _CACHED_NC, _CACHED_BIAS, LAST_EXEC_NS
    x = np.asarray(x, dtype=np.float32)
    W = np.asarray(W, dtype=np.float32)
    b = np.asarray(b, dtype=np.float32)
    mask = np.asarray(neighbor_mask)

    has_bias = bool(np.any(b))
    if _CACHED_NC is None or _CACHED_BIAS != has_bias:
        _CACHED_NC = _build_nc(has_bias=has_bias)
        _CACHED_BIAS = has_bias
    nc = _CACHED_NC

    mask_u8 = mask.astype(np.uint8)
    in_maps = []
    for core in range(N_CORES):
        h, rb = core // 2, core % 2
        r0 = rb * ROWS
        xt = np.ascontiguousarray(x[h, r0:r0 + ROWS].T).astype(BF_NP)
        wt = np.ascontiguousarray(W[h].T).astype(BF_NP)
        xa = np.concatenate(
            [x[h], np.ones((N, 1), np.float32)], axis=1
        ).astype(BF_NP)
        mk = _pack_mask(
            np.ascontiguousarray(mask_u8[r0:r0 + ROWS].T), has_bias)
        bc = np.ascontiguousarray(b[h].reshape(M_TILES, 128).T)
        in_maps.append({"xt": xt, "wt": wt, "xa": xa, "mk": mk, "bc": bc})

    res = run_bass_kernel_spmd(nc, in_maps, core_ids=list(range(N_CORES)),
                               trace=TRACE)
    LAST_EXEC_NS = res.exec_time_ns

    out = np.empty((H, N, D), np.float32)
    for core in range(N_CORES):
        h, rb = core // 2, core % 2
        r0 = rb * ROWS
        out[h, r0:r0 + ROWS] = res.results[core]["out"]
    return out


# revision 40
# speedup vs baseline: 1.1037x; 1.1037x over previous
"""Distributed Trainium2 (Bass/Tile) kernel for masked GAT-style attention.

Reference computation (H=4 heads, N=4096 nodes, D=128):
    scores = leaky_relu(x @ W^T + b, 0.2)            # [H, N, N]
    att    = where(mask, softmax(where(mask, scores, -inf)), 0)
    out    = att @ x                                  # [H, N, D]

Sharding: 8 cores = 4 heads x 2 row-blocks of 2048 nodes. Each core
computes out[h, r0:r0+2048] independently (no collectives).

Per-core layout ("transposed scores"): scores^T tiles [m=128 part, n free]
so the PV matmul uses the attention tile directly as the stationary
operand and the softmax row-sum comes for free from an appended
ones-column on x.

The per-element work is split across engines by m-tile:
- ACT path (m % 3 == 0): leaky_relu via the Prelu LUT, exp, then a
  bf16 mask multiply on VectorE (2x mode).
- DVE path (otherwise): one scalar_tensor_tensor computes
  t = 0.2*s + (-30)*(1-mask) (the additive term folds the mask in:
  masked-out entries end up as exp(0.2*s - 30) ~ 1e-13), a second STT
  computes l = max(5t, t) = leaky(s) - 150*(1-mask), then exp on ACT.
  No post-exp mask multiply needed.
The mask is shipped once as uint16: bf16 1.0/0.0 rows for ACT-path
tiles, fp16 -30*(1-mask) rows for DVE-path tiles, bitcast on chip.
"""

import sys

if "/opt/trn_rl_repo" not in sys.path:
    sys.path.insert(0, "/opt/trn_rl_repo")

import numpy as np
import ml_dtypes

import concourse.bass as bass
import concourse.tile as tile
from concourse import bacc, mybir
from concourse.bass_utils import run_bass_kernel_spmd

BF = mybir.dt.bfloat16
F16 = mybir.dt.float16
F32 = mybir.dt.float32
U16 = mybir.dt.uint16
BF_NP = ml_dtypes.bfloat16

H, N, D = 4, 4096, 128
N_CORES = 8
ROWS = N * H // N_CORES          # 2048 rows (n) per core
CHUNK = 1024                     # n columns processed per outer chunk
CHUNKS = ROWS // CHUNK           # 2
M_TILES = N // 128               # 32 tiles along the softmax (m) axis
SUBS = CHUNK // 128              # 8 PV subtiles per chunk
M_GROUP = 8                      # m-tiles per split constant tile
MB = 4                           # m-tiles per batched mask DMA

# Module-level knobs used by test.py; harmless defaults for grading.
TRACE = False
LAST_EXEC_NS = None

_CACHED_NC = None
_CACHED_BIAS = None


def _on_act(m, has_bias):
    return has_bias or m % 2 == 0


def _build_nc(has_bias=False):
    nc = bacc.Bacc("TRN2", target_bir_lowering=False, debug=False,
                   num_devices=N_CORES)
    xt_d = nc.dram_tensor("xt", [128, ROWS], BF, kind="ExternalInput").ap()
    wt_d = nc.dram_tensor("wt", [128, N], BF, kind="ExternalInput").ap()
    xa_d = nc.dram_tensor("xa", [N, D + 1], BF, kind="ExternalInput").ap()
    mk_d = nc.dram_tensor("mk", [N, ROWS], U16, kind="ExternalInput").ap()
    bc_d = nc.dram_tensor("bc", [128, M_TILES], F32, kind="ExternalInput").ap()
    out_d = nc.dram_tensor("out", [ROWS, D], F32, kind="ExternalOutput").ap()

    PRELU = mybir.ActivationFunctionType.Prelu
    EXP = mybir.ActivationFunctionType.Exp
    COPY = mybir.ActivationFunctionType.Copy
    MUL = mybir.AluOpType.mult
    ADD = mybir.AluOpType.add
    MAX = mybir.AluOpType.max

    n_wt = N // (M_GROUP * 128)      # 4 stationary groups
    n_xa = M_TILES // M_GROUP        # 4 PV-moving groups

    with tile.TileContext(nc) as tc:
        with (
            tc.tile_pool(name="const", bufs=1) as cpool,
            tc.tile_pool(name="mask", bufs=3) as mpool,
            tc.tile_pool(name="work", bufs=6) as wpool,
            tc.tile_pool(name="outp", bufs=3) as opool,
            tc.tile_pool(name="spsum", bufs=2, space="PSUM") as spool,
            tc.tile_pool(name="opsum", bufs=1, space="PSUM") as oppool,
        ):
            # Constants, split into group tiles so the first QK/PV tiles
            # only depend on the first pieces.
            GW = M_GROUP * 128
            wt_sb = [cpool.tile([128, GW], BF, name=f"wt{i}")
                     for i in range(n_wt)]
            xt_sb = [cpool.tile([128, CHUNK], BF, name=f"xt{i}")
                     for i in range(CHUNKS)]
            bc_sb = cpool.tile([128, M_TILES], F32)
            xa_sb = [cpool.tile([128, M_GROUP, D + 1], BF, name=f"xa{i}")
                     for i in range(n_xa)]

            # Warm the ACT LUT set while input DMAs are in flight: a tiny
            # dependency-free activation forces the table load up front.
            warm_sb = cpool.tile([1, 1], F32)
            nc.scalar.activation(warm_sb[:], warm_sb[:], EXP)

            nc.sync.dma_start(out=wt_sb[0][:], in_=wt_d[:, 0:GW])
            nc.sync.dma_start(out=xt_sb[0][:], in_=xt_d[:, 0:CHUNK])
            nc.sync.dma_start(out=bc_sb[:], in_=bc_d[:, :])
            # xa: one batched DMA per group, issued from the idle GpSimd
            # sequencer so they don't delay mask issues on Sync.
            xa_v = xa_d.rearrange("(g j p) d -> g p j d", p=128, j=M_GROUP)
            for i in range(1, n_wt):
                nc.sync.dma_start(out=wt_sb[i][:], in_=wt_d[:, i * GW:(i + 1) * GW])
            for i in range(1, CHUNKS):
                nc.sync.dma_start(out=xt_sb[i][:],
                                  in_=xt_d[:, i * CHUNK:(i + 1) * CHUNK])

            for c in range(CHUNKS):
                # 8 accumulator subtiles of [128, 129] packed 3-per-bank.
                o_ps = [
                    oppool.tile([128, 512], F32, tag=f"oacc{b}",
                                name=f"oacc{b}_c{c}")
                    for b in range((SUBS + 2) // 3)
                ]

                def o_ap(s):
                    return o_ps[s // 3][:, (s % 3) * 129:(s % 3) * 129 + 129]

                mkb_sb = None
                for pair in range(M_TILES // 2):
                    pv_lhs = {}
                    lp_sb = wpool.tile([128, 2, CHUNK], F16, tag="lp",
                                       name=f"lp_c{c}_p{pair}")
                    for half_idx in range(2):
                        m = pair * 2 + half_idx
                        # Mask tiles: one batched DMA per MB m-tiles, issued
                        # from the GpSimd sequencer (Sync is saturated with
                        # per-tile issue otherwise).
                        if m % MB == 0:
                            mkb_sb = mpool.tile([128, MB, CHUNK], U16,
                                                tag="mkb",
                                                name=f"mkb_c{c}_m{m}")
                            mk_v = mk_d[m * 128:(m + MB) * 128,
                                        c * CHUNK:(c + 1) * CHUNK].rearrange(
                                            "(t p) n -> p t n", p=128)
                            nc.gpsimd.dma_start(out=mkb_sb[:], in_=mk_v)
                            if c == 0 and m == 0:
                                for g in range(n_xa):
                                    nc.gpsimd.dma_start(out=xa_sb[g][:],
                                                        in_=xa_v[g])
                        mk_sb = mkb_sb[:, m % MB]

                        # scores^T tile: [m=128, n=CHUNK] (two 512-col
                        # matmuls, one per PSUM bank).
                        s_ps = spool.tile([128, CHUNK], F32, tag="s",
                                          name=f"s_c{c}_m{m}")
                        wt_g = wt_sb[m // M_GROUP]
                        wcol = (m % M_GROUP) * 128
                        for half in range(CHUNK // 512):
                            nc.tensor.matmul(
                                s_ps[:, half * 512:(half + 1) * 512],
                                lhsT=wt_g[:, wcol:wcol + 128],
                                rhs=xt_sb[c][:, half * 512:(half + 1) * 512],
                                start=True, stop=True,
                            )

                        l_half = lp_sb[:, half_idx]
                        if _on_act(m, has_bias):
                            # ACT path: Prelu LUT (+ per-partition bias);
                            # the bf16 mask multiply happens after exp.
                            nc.scalar.activation(l_half, s_ps[:], PRELU,
                                                 bias=bc_sb[:, m:m + 1],
                                                 scale=1.0, alpha=0.2)
                            pv_lhs[half_idx] = ("mask", mk_sb)
                        else:
                            # DVE path: t = 0.2*s + (-30)*(1-mask)  [one
                            # STT, single PSUM read], l = max(5t, t)  [STT
                            # in fp16 SBUF]. Mask already folded in.
                            t_sb = wpool.tile([128, CHUNK], F16, tag="lv_t",
                                              name=f"t_c{c}_m{m}")
                            nc.vector.scalar_tensor_tensor(
                                t_sb[:], s_ps[:], 0.2, mk_sb.bitcast(F16),
                                op0=MUL, op1=ADD)
                            nc.vector.scalar_tensor_tensor(
                                l_half, t_sb[:], 5.0, t_sb[:],
                                op0=MUL, op1=MAX)
                            pv_lhs[half_idx] = ("plain", None)

                    # One exp covers both halves of the pair.
                    ep_sb = wpool.tile([128, 2, CHUNK], BF, tag="ep",
                                       name=f"ep_c{c}_p{pair}")
                    nc.scalar.activation(ep_sb[:], lp_sb[:], EXP)

                    for half_idx in range(2):
                        m = pair * 2 + half_idx
                        kind, mk_sb2 = pv_lhs[half_idx]
                        if kind == "mask":
                            a_sb = wpool.tile([128, CHUNK], BF, tag="a",
                                              name=f"a_c{c}_m{m}")
                            nc.vector.tensor_mul(a_sb[:], ep_sb[:, half_idx],
                                                 mk_sb2.bitcast(BF))
                            lhs_t = a_sb[:]
                        else:
                            lhs_t = ep_sb[:, half_idx]

                        # PV: out[n, 0:128] += att^T.T @ x ; col 128 =
                        # row-sum. start=True clears has_written for the
                        # WHOLE bank, so only the first sub-chain of each
                        # bank may issue it; later sub-ranges land via
                        # per-element overwrite-on-first-write semantics.
                        for s in range(SUBS):
                            nc.tensor.matmul(
                                o_ap(s),
                                lhsT=lhs_t[:, s * 128:(s + 1) * 128],
                                rhs=xa_sb[m // M_GROUP][:, m % M_GROUP],
                                start=(m == 0 and s % 3 == 0),
                                stop=(m == M_TILES - 1),
                                skip_group_check=True,
                            )

                # Division tail: reciprocal on DVE (tiny), the scale-copy
                # split across ACT and DVE, all results gathered into one
                # tile so the chunk needs a single output DMA.
                of_big = opool.tile([128, SUBS, D], F32, tag="ofbig",
                                    name=f"ofbig_c{c}")
                for s in range(SUBS):
                    ob = o_ap(s)
                    r_sb = opool.tile([128, 1], F32, tag="recip",
                                      name=f"recip_c{c}_s{s}")
                    nc.vector.reciprocal(r_sb[:], ob[:, 128:129])
                    if s % 2 == 0:
                        nc.scalar.activation(of_big[:, s], ob[:, 0:D], COPY,
                                             bias=0.0, scale=r_sb[:])
                    else:
                        nc.vector.tensor_scalar_mul(of_big[:, s], ob[:, 0:D],
                                                    r_sb[:])
                out_v = out_d[c * CHUNK:(c + 1) * CHUNK, :].rearrange(
                    "(s p) d -> p s d", p=128)
                nc.sync.dma_start(out=out_v, in_=of_big[:])

    nc.compile()
    return nc


def _pack_mask(mask_t_u8, has_bias):
    """mask_t_u8: [N, ROWS] 0/1. Returns uint16-packed per-m-tile rows."""
    out = np.empty(mask_t_u8.shape, np.uint16)
    for m in range(M_TILES):
        rows = slice(m * 128, (m + 1) * 128)
        blk = mask_t_u8[rows]
        if _on_act(m, has_bias):
            out[rows] = blk.astype(BF_NP).view(np.uint16)
        else:
            out[rows] = (30.0 * (blk.astype(np.float32) - 1.0)).astype(
                np.float16).view(np.uint16)
    return out


def kernel(x, W, b, neighbor_mask):
    global _CACHED_NC, _CACHED_BIAS, LAST_EXEC_NS
    x = np.asarray(x, dtype=np.float32)
    W = np.asarray(W, dtype=np.float32)
    b = np.asarray(b, dtype=np.float32)
    mask = np.asarray(neighbor_mask)

    has_bias = bool(np.any(b))
    if _CACHED_NC is None or _CACHED_BIAS != has_bias:
        _CACHED_NC = _build_nc(has_bias=has_bias)
        _CACHED_BIAS = has_bias
    nc = _CACHED_NC

    mask_u8 = mask.astype(np.uint8)
    in_maps = []
    for core in range(N_CORES):
        h, rb = core // 2, core % 2
        r0 = rb * ROWS
        xt = np.ascontiguousarray(x[h, r0:r0 + ROWS].T).astype(BF_NP)
        wt = np.ascontiguousarray(W[h].T).astype(BF_NP)
        xa = np.concatenate(
            [x[h], np.ones((N, 1), np.float32)], axis=1
        ).astype(BF_NP)
        mk = _pack_mask(
            np.ascontiguousarray(mask_u8[r0:r0 + ROWS].T), has_bias)
        bc = np.ascontiguousarray(b[h].reshape(M_TILES, 128).T)
        in_maps.append({"xt": xt, "wt": wt, "xa": xa, "mk": mk, "bc": bc})

    res = run_bass_kernel_spmd(nc, in_maps, core_ids=list(range(N_CORES)),
                               trace=TRACE)
    LAST_EXEC_NS = res.exec_time_ns

    out = np.empty((H, N, D), np.float32)
    for core in range(N_CORES):
        h, rb = core // 2, core % 2
        r0 = rb * ROWS
        out[h, r0:r0 + ROWS] = res.results[core]["out"]
    return out


# revision 41
# speedup vs baseline: 1.1207x; 1.0154x over previous
"""Distributed Trainium2 (Bass/Tile) kernel for masked GAT-style attention.

Reference computation (H=4 heads, N=4096 nodes, D=128):
    scores = leaky_relu(x @ W^T + b, 0.2)            # [H, N, N]
    att    = where(mask, softmax(where(mask, scores, -inf)), 0)
    out    = att @ x                                  # [H, N, D]

Sharding: 8 cores = 4 heads x 2 row-blocks of 2048 nodes. Each core
computes out[h, r0:r0+2048] independently (no collectives).

Per-core layout ("transposed scores"): scores^T tiles [m=128 part, n free]
so the PV matmul uses the attention tile directly as the stationary
operand and the softmax row-sum comes for free from an appended
ones-column on x.

The per-element work is split across engines by m-tile:
- ACT path (m % 3 == 0): leaky_relu via the Prelu LUT, exp, then a
  bf16 mask multiply on VectorE (2x mode).
- DVE path (otherwise): one scalar_tensor_tensor computes
  t = 0.2*s + (-30)*(1-mask) (the additive term folds the mask in:
  masked-out entries end up as exp(0.2*s - 30) ~ 1e-13), a second STT
  computes l = max(5t, t) = leaky(s) - 150*(1-mask), then exp on ACT.
  No post-exp mask multiply needed.
The mask is shipped once as uint16: bf16 1.0/0.0 rows for ACT-path
tiles, fp16 -30*(1-mask) rows for DVE-path tiles, bitcast on chip.
"""

import sys

if "/opt/trn_rl_repo" not in sys.path:
    sys.path.insert(0, "/opt/trn_rl_repo")

import numpy as np
import ml_dtypes

import concourse.bass as bass
import concourse.tile as tile
from concourse import bacc, mybir
from concourse.bass_utils import run_bass_kernel_spmd

BF = mybir.dt.bfloat16
F16 = mybir.dt.float16
F32 = mybir.dt.float32
U16 = mybir.dt.uint16
BF_NP = ml_dtypes.bfloat16

H, N, D = 4, 4096, 128
N_CORES = 8
ROWS = N * H // N_CORES          # 2048 rows (n) per core
CHUNK = 1024                     # n columns processed per outer chunk
CHUNKS = ROWS // CHUNK           # 2
M_TILES = N // 128               # 32 tiles along the softmax (m) axis
SUBS = CHUNK // 128              # 8 PV subtiles per chunk
M_GROUP = 8                      # m-tiles per split constant tile
MB = 4                           # m-tiles per batched mask DMA

# Module-level knobs used by test.py; harmless defaults for grading.
TRACE = False
LAST_EXEC_NS = None

_CACHED_NC = None
_CACHED_BIAS = None


def _on_act(m, has_bias):
    return has_bias or m % 2 == 0


def _build_nc(has_bias=False):
    nc = bacc.Bacc("TRN2", target_bir_lowering=False, debug=False,
                   num_devices=N_CORES)
    xt_d = nc.dram_tensor("xt", [128, ROWS], BF, kind="ExternalInput").ap()
    wt_d = nc.dram_tensor("wt", [128, N], BF, kind="ExternalInput").ap()
    xa_d = nc.dram_tensor("xa", [N, D + 1], BF, kind="ExternalInput").ap()
    mk_d = nc.dram_tensor("mk", [N, ROWS], U16, kind="ExternalInput").ap()
    bc_d = nc.dram_tensor("bc", [128, M_TILES], F32, kind="ExternalInput").ap()
    out_d = nc.dram_tensor("out", [ROWS, D], F32, kind="ExternalOutput").ap()

    PRELU = mybir.ActivationFunctionType.Prelu
    EXP = mybir.ActivationFunctionType.Exp
    COPY = mybir.ActivationFunctionType.Copy
    MUL = mybir.AluOpType.mult
    ADD = mybir.AluOpType.add
    MAX = mybir.AluOpType.max

    n_wt = N // (M_GROUP * 128)      # 4 stationary groups
    n_xa = M_TILES // M_GROUP        # 4 PV-moving groups

    with tile.TileContext(nc) as tc:
        with (
            tc.tile_pool(name="const", bufs=1) as cpool,
            tc.tile_pool(name="mask", bufs=4) as mpool,
            tc.tile_pool(name="work", bufs=8) as wpool,
            tc.tile_pool(name="outp", bufs=3) as opool,
            tc.tile_pool(name="spsum", bufs=2, space="PSUM") as spool,
            tc.tile_pool(name="opsum", bufs=1, space="PSUM") as oppool,
        ):
            # Constants, split into group tiles so the first QK/PV tiles
            # only depend on the first pieces.
            GW = M_GROUP * 128
            wt_sb = [cpool.tile([128, GW], BF, name=f"wt{i}")
                     for i in range(n_wt)]
            xt_sb = [cpool.tile([128, CHUNK], BF, name=f"xt{i}")
                     for i in range(CHUNKS)]
            bc_sb = cpool.tile([128, M_TILES], F32)
            xa_sb = [cpool.tile([128, M_GROUP, D + 1], BF, name=f"xa{i}")
                     for i in range(n_xa)]

            # Warm the ACT LUT set while input DMAs are in flight: a tiny
            # dependency-free activation forces the table load up front.
            warm_sb = cpool.tile([1, 1], F32)
            nc.scalar.activation(warm_sb[:], warm_sb[:], EXP)

            nc.sync.dma_start(out=wt_sb[0][:], in_=wt_d[:, 0:GW])
            nc.sync.dma_start(out=xt_sb[0][:], in_=xt_d[:, 0:CHUNK])
            nc.sync.dma_start(out=bc_sb[:], in_=bc_d[:, :])
            # xa: one batched DMA per group, issued from the idle GpSimd
            # sequencer so they don't delay mask issues on Sync.
            xa_v = xa_d.rearrange("(g j p) d -> g p j d", p=128, j=M_GROUP)
            for i in range(1, n_wt):
                nc.sync.dma_start(out=wt_sb[i][:], in_=wt_d[:, i * GW:(i + 1) * GW])
            for i in range(1, CHUNKS):
                nc.sync.dma_start(out=xt_sb[i][:],
                                  in_=xt_d[:, i * CHUNK:(i + 1) * CHUNK])

            for c in range(CHUNKS):
                # 8 accumulator subtiles of [128, 129] packed 3-per-bank.
                o_ps = [
                    oppool.tile([128, 512], F32, tag=f"oacc{b}",
                                name=f"oacc{b}_c{c}")
                    for b in range((SUBS + 2) // 3)
                ]

                def o_ap(s):
                    return o_ps[s // 3][:, (s % 3) * 129:(s % 3) * 129 + 129]

                mkb_sb = None
                for pair in range(M_TILES // 2):
                    pv_lhs = {}
                    lp_sb = wpool.tile([128, 2, CHUNK], F16, tag="lp",
                                       name=f"lp_c{c}_p{pair}")
                    for half_idx in range(2):
                        m = pair * 2 + half_idx
                        # Mask tiles: one batched DMA per MB m-tiles, issued
                        # from the GpSimd sequencer (Sync is saturated with
                        # per-tile issue otherwise).
                        if m % MB == 0:
                            mkb_sb = mpool.tile([128, MB, CHUNK], U16,
                                                tag="mkb",
                                                name=f"mkb_c{c}_m{m}")
                            mk_v = mk_d[m * 128:(m + MB) * 128,
                                        c * CHUNK:(c + 1) * CHUNK].rearrange(
                                            "(t p) n -> p t n", p=128)
                            nc.gpsimd.dma_start(out=mkb_sb[:], in_=mk_v)
                            if c == 0 and m == 0:
                                for g in range(n_xa):
                                    nc.gpsimd.dma_start(out=xa_sb[g][:],
                                                        in_=xa_v[g])
                        mk_sb = mkb_sb[:, m % MB]

                        # scores^T tile: [m=128, n=CHUNK] (two 512-col
                        # matmuls, one per PSUM bank).
                        s_ps = spool.tile([128, CHUNK], F32, tag="s",
                                          name=f"s_c{c}_m{m}")
                        wt_g = wt_sb[m // M_GROUP]
                        wcol = (m % M_GROUP) * 128
                        for half in range(CHUNK // 512):
                            nc.tensor.matmul(
                                s_ps[:, half * 512:(half + 1) * 512],
                                lhsT=wt_g[:, wcol:wcol + 128],
                                rhs=xt_sb[c][:, half * 512:(half + 1) * 512],
                                start=True, stop=True,
                            )

                        l_half = lp_sb[:, half_idx]
                        if _on_act(m, has_bias):
                            # ACT path: Prelu LUT (+ per-partition bias);
                            # the bf16 mask multiply happens after exp.
                            nc.scalar.activation(l_half, s_ps[:], PRELU,
                                                 bias=bc_sb[:, m:m + 1],
                                                 scale=1.0, alpha=0.2)
                            pv_lhs[half_idx] = ("mask", mk_sb)
                        else:
                            # DVE path: t = 0.2*s + (-30)*(1-mask)  [one
                            # STT, single PSUM read], l = max(5t, t)  [STT
                            # in fp16 SBUF]. Mask already folded in.
                            t_sb = wpool.tile([128, CHUNK], F16, tag="lv_t",
                                              name=f"t_c{c}_m{m}")
                            nc.vector.scalar_tensor_tensor(
                                t_sb[:], s_ps[:], 0.2, mk_sb.bitcast(F16),
                                op0=MUL, op1=ADD)
                            nc.vector.scalar_tensor_tensor(
                                l_half, t_sb[:], 5.0, t_sb[:],
                                op0=MUL, op1=MAX)
                            pv_lhs[half_idx] = ("plain", None)

                    # One exp covers both halves of the pair.
                    ep_sb = wpool.tile([128, 2, CHUNK], BF, tag="ep",
                                       name=f"ep_c{c}_p{pair}")
                    nc.scalar.activation(ep_sb[:], lp_sb[:], EXP)

                    for half_idx in range(2):
                        m = pair * 2 + half_idx
                        kind, mk_sb2 = pv_lhs[half_idx]
                        if kind == "mask":
                            a_sb = wpool.tile([128, CHUNK], BF, tag="a",
                                              name=f"a_c{c}_m{m}")
                            nc.vector.tensor_mul(a_sb[:], ep_sb[:, half_idx],
                                                 mk_sb2.bitcast(BF))
                            lhs_t = a_sb[:]
                        else:
                            lhs_t = ep_sb[:, half_idx]

                        # PV: out[n, 0:128] += att^T.T @ x ; col 128 =
                        # row-sum. start=True clears has_written for the
                        # WHOLE bank, so only the first sub-chain of each
                        # bank may issue it; later sub-ranges land via
                        # per-element overwrite-on-first-write semantics.
                        for s in range(SUBS):
                            nc.tensor.matmul(
                                o_ap(s),
                                lhsT=lhs_t[:, s * 128:(s + 1) * 128],
                                rhs=xa_sb[m // M_GROUP][:, m % M_GROUP],
                                start=(m == 0 and s % 3 == 0),
                                stop=(m == M_TILES - 1),
                                skip_group_check=True,
                            )

                # Division tail: reciprocal on DVE (tiny), the scale-copy
                # split across ACT and DVE, all results gathered into one
                # tile so the chunk needs a single output DMA.
                of_big = opool.tile([128, SUBS, D], F32, tag="ofbig",
                                    name=f"ofbig_c{c}")
                for s in range(SUBS):
                    ob = o_ap(s)
                    r_sb = opool.tile([128, 1], F32, tag="recip",
                                      name=f"recip_c{c}_s{s}")
                    nc.vector.reciprocal(r_sb[:], ob[:, 128:129])
                    if s % 2 == 0:
                        nc.scalar.activation(of_big[:, s], ob[:, 0:D], COPY,
                                             bias=0.0, scale=r_sb[:])
                    else:
                        nc.vector.tensor_scalar_mul(of_big[:, s], ob[:, 0:D],
                                                    r_sb[:])
                out_v = out_d[c * CHUNK:(c + 1) * CHUNK, :].rearrange(
                    "(s p) d -> p s d", p=128)
                nc.sync.dma_start(out=out_v, in_=of_big[:])

    nc.compile()
    return nc


def _pack_mask(mask_t_u8, has_bias):
    """mask_t_u8: [N, ROWS] 0/1. Returns uint16-packed per-m-tile rows."""
    out = np.empty(mask_t_u8.shape, np.uint16)
    for m in range(M_TILES):
        rows = slice(m * 128, (m + 1) * 128)
        blk = mask_t_u8[rows]
        if _on_act(m, has_bias):
            out[rows] = blk.astype(BF_NP).view(np.uint16)
        else:
            out[rows] = (30.0 * (blk.astype(np.float32) - 1.0)).astype(
                np.float16).view(np.uint16)
    return out


def kernel(x, W, b, neighbor_mask):
    global _CACHED_NC, _CACHED_BIAS, LAST_EXEC_NS
    x = np.asarray(x, dtype=np.float32)
    W = np.asarray(W, dtype=np.float32)
    b = np.asarray(b, dtype=np.float32)
    mask = np.asarray(neighbor_mask)

    has_bias = bool(np.any(b))
    if _CACHED_NC is None or _CACHED_BIAS != has_bias:
        _CACHED_NC = _build_nc(has_bias=has_bias)
        _CACHED_BIAS = has_bias
    nc = _CACHED_NC

    mask_u8 = mask.astype(np.uint8)
    in_maps = []
    for core in range(N_CORES):
        h, rb = core // 2, core % 2
        r0 = rb * ROWS
        xt = np.ascontiguousarray(x[h, r0:r0 + ROWS].T).astype(BF_NP)
        wt = np.ascontiguousarray(W[h].T).astype(BF_NP)
        xa = np.concatenate(
            [x[h], np.ones((N, 1), np.float32)], axis=1
        ).astype(BF_NP)
        mk = _pack_mask(
            np.ascontiguousarray(mask_u8[r0:r0 + ROWS].T), has_bias)
        bc = np.ascontiguousarray(b[h].reshape(M_TILES, 128).T)
        in_maps.append({"xt": xt, "wt": wt, "xa": xa, "mk": mk, "bc": bc})

    res = run_bass_kernel_spmd(nc, in_maps, core_ids=list(range(N_CORES)),
                               trace=TRACE)
    LAST_EXEC_NS = res.exec_time_ns

    out = np.empty((H, N, D), np.float32)
    for core in range(N_CORES):
        h, rb = core // 2, core % 2
        r0 = rb * ROWS
        out[h, r0:r0 + ROWS] = res.results[core]["out"]
    return out
